# revision 17
# baseline (speedup 1.0000x reference)
"""Trainium2 Bass kernel for the soft-DTW shape+temporal loss.

Problem: input/target (4, 128, 16, 4, 4) = (B, T, C, H, W). Each of the
B*C*H*W = 1024 spatial cells is an independent univariate series of length
T = 128. Per series: squared-L2 cost matrix D, soft-DTW forward DP value
R[N,N] (loss_shape), soft alignment path = dR[N,N]/dD (via the standard
backward E-recursion), temporal loss = sum(path * Omega)/T^2 with
Omega[i,j] = (i-j)^2.

Sharding: 1024 series / 8 cores = 128 series per core, one series per SBUF
partition. The DP runs along anti-diagonals (wavefront); diagonal cells are
laid out along the free dimension, so every wavefront step is a handful of
full-width vector ops. R is stored diagonal-major (slot width DL per
diagonal) for the whole forward sweep; the backward pass re-reads it.

Cost-matrix diagonals are precomputed in windows of W=8 diagonals with one
sheared 2D-access-pattern subtract + one sheared Square activation (instead
of two per-step ops); post-exp quantities (softmin exp terms, path weights,
E) are bf16 so their sums run in the DVE's 2x packed mode.

Host side shards the inputs, runs the same program SPMD on 8 cores, and
reduces the per-series results to the 3 scalar losses.
"""

import sys

for _p in ("/opt/trn_rl_repo",):
    if _p not in sys.path:
        sys.path.insert(0, _p)

import numpy as np

import concourse.bass as bass
import concourse.mybir as mybir
from concourse import bass_utils
from concourse.tile import TileContext

# ---- problem constants (hardcoded per contract) ----
B, T, C, H, W_ = 4, 128, 16, 4, 4
N = T
NCORES = 8
SPC = (B * C * H * W_) // NCORES  # 128 series per core
ALPHA = 0.5
GAMMA = 0.01
INVG = 1.0 / GAMMA
BIG = 1e8
SENT = 1.0e6  # sentinel pad; (x - SENT)^2 ~ 1e12 >> BIG kills boundary weights

DL = 140  # per-diagonal slot width (>= N + DW + 4 so window tails stay in-slot)
ND = 2 * N + 1  # diagonals 0..2N
KS = 4  # S ring depth
KE = 4  # E ring depth
SQOFF = 256  # sqtab column offset: col = (2*idx - d) + SQOFF
DW = 8  # D-precompute window size (diagonals per window)
TP = 140  # padded width of t/p_rev host arrays (sentinel tail for windows)

F32 = mybir.dt.float32
BF16 = mybir.dt.bfloat16
I32 = mybir.dt.int32
AF = mybir.ActivationFunctionType
OP = mybir.AluOpType


def _rng(d):
    """Valid idx range [lo, hi] of diagonal d (cells (i=idx, j=d-idx))."""
    return max(1, d - N), min(N, d - 1)


def _win_list():
    """Windows of diagonals, each entirely in the lower (d<=N+1) or upper
    half so the sheared access patterns stay affine."""
    wins = []
    d = 2
    while d <= N + 1:
        wins.append((d, min(d + DW - 1, N + 1)))
        d += DW
    d = N + 2
    while d <= 2 * N:
        wins.append((d, min(d + DW - 1, 2 * N)))
        d += DW
    return wins


def _split_multi_waits(nc):
    """walrus here rejects >1 sync wait per TPB instruction.

    Pass 1 (ACT only): drop self-engine waits that are provably satisfied
    by program order — the ACT instruction struct cannot carry 2 waits and
    NoOp carriers are rejected by the ACT codegen path.
    Pass 2: hoist remaining extra waits onto same-engine NoOp carriers.
    """
    pre_of = {
        mybir.EngineType.DVE: "DVE",
        mybir.EngineType.Activation: "Activation",
        mybir.EngineType.Pool: "Pool",
        mybir.EngineType.SP: "SP",
        mybir.EngineType.PE: "PE",
    }
    nsplit = 0
    inc = {}  # (engine, sem id) -> inc count so far, in block order
    tainted = set()
    for f in nc.m.functions:
        for bb in f.blocks:
            insts = list(bb.instructions)
            new = []
            changed = False
            for ins in insts:
                si = ins.sync_info
                eng = ins.engine
                pre = pre_of.get(eng)
                waits = list(si.on_wait) if si is not None and si.on_wait else []
                if (
                    waits
                    and pre is not None
                    and len(waits) > 1
                    and eng == mybir.EngineType.Activation
                ):
                    keep = [
                        w
                        for w in waits
                        if not (
                            w.sync_type == "semaphore"
                            and w.wait_mode == "sem-ge-imm"
                            and w.ant_name
                            and w.ant_name.split("_")[0] == pre
                            and w.id not in tainted
                            and w.wait_value <= inc.get((eng, w.id), 0)
                        )
                    ]
                else:
                    keep = waits
                if len(keep) > 1:
                    for w in keep[:-1]:
                        nsplit += 1
                        new.append(
                            mybir.InstNoOp(
                                name=f"wsplit-{nsplit}",
                                engine=eng,
                                sync_info=mybir.SyncInfo(on_wait=[w], on_update=[]),
                            )
                        )
                    keep = [keep[-1]]
                    changed = True
                if si is not None and len(keep) != len(waits):
                    ins.sync_info = mybir.SyncInfo(
                        on_wait=keep, on_update=list(si.on_update or [])
                    )
                    changed = True
                if si is not None and si.on_update:
                    for u in si.on_update:
                        if u.update_mode == "sem-inc":
                            inc[(eng, u.id)] = inc.get((eng, u.id), 0) + (
                                u.update_value or 0
                            )
                        else:
                            tainted.add(u.id)
                new.append(ins)
            if changed:
                bb.instructions = new
    return nsplit


def _mk_ap(tile_ap, off, axes):
    """Raw AP over a tile: axes = [[stride, count], ...] after the partition
    axis (which is taken from the tile)."""
    base = tile_ap[:, 0:1]
    return bass.AP(
        tensor=base.tensor, offset=off, ap=[[base.ap[0][0], SPC]] + axes
    )


def build_nc(legalize=True):
    nc = bass.Bass("TRN2", debug=False, num_devices=NCORES)
    t_ext_d = nc.dram_tensor("t_ext", [SPC, TP], F32, kind="ExternalInput")
    p_rev_d = nc.dram_tensor("p_rev_ext", [SPC, TP], F32, kind="ExternalInput")
    out_d = nc.dram_tensor("out", [SPC, 2], F32, kind="ExternalOutput")

    wins = _win_list()
    NW = len(wins)
    NB = 2 * N - 1 - 1  # number of backward steps (d = 2N-1 .. 2)

    with TileContext(nc) as tc:
        with tc.tile_pool(name="main", bufs=1) as pool:
            v = nc.vector
            s = nc.scalar

            # ---- persistent state ----
            R = pool.tile([SPC, ND * DL], F32, tag="R")
            text = pool.tile([SPC, TP], F32, tag="text")
            prev = pool.tile([SPC, TP], F32, tag="prev")
            sqi = pool.tile([SPC, 512], I32, tag="sqi")
            sqt = pool.tile([SPC, 512], F32, tag="sqt")
            Sr = pool.tile([SPC, KS * DL], F32, tag="Sr")
            Er = pool.tile([SPC, KE * DL], BF16, tag="Er")
            accv = pool.tile([SPC, 256], F32, tag="accv")
            accs = pool.tile([SPC, 1], F32, tag="accs")
            outp = pool.tile([SPC, 2], F32, tag="outp")

            nc.sync.dma_start(text[:, :], t_ext_d[:, :])
            nc.sync.dma_start(prev[:, :], p_rev_d[:, :])

            # sq table: sqt[col] = (col - SQOFF)^2, same in every partition
            nc.gpsimd.iota(sqi[:, :], pattern=[[1, 512]], base=0, channel_multiplier=0)
            nbias = pool.tile([SPC, 1], F32, tag="nbias")
            nc.gpsimd.memset(nbias[:, :], float(-SQOFF))
            s.activation(sqt[:, :], sqi[:, :], AF.Square, bias=nbias[:, 0:1])

            # ---- R boundary init (only slots ever read as BIG) ----
            v.memset(R[:, 1 : N + 2], BIG)
            v.memset(R[:, DL : DL + N + 2], BIG)  # diag 1
            v.memset(R[:, 0:1], 0.0)
            # column 0 of diags 2..N+1 (lo-1 boundary, lower half)
            v.memset(R[:, 2 * DL : (N + 2) * DL : DL], BIG)
            # lo-1 boundary, upper half: diag d >= N+2 at position d-N-1
            v.memset(R[:, (N + 2) * DL + 1 : ND * DL : DL + 1], BIG)
            # hi+1 boundary, lower half: diag d in 1..N at position d
            v.memset(R[:, (DL + 1) : (N + 1) * (DL + 1) : DL + 1], BIG)
            # hi+1 boundary, upper half: diag d >= N+1 at position N+1
            v.memset(R[:, (N + 1) * DL + N + 1 : ND * DL : DL], BIG)

            v.memset(Sr[:, :], -BIG)
            # E ring: all zeros except E[2N][N] = 1 (disjoint writes)
            e1 = ((2 * N) % KE) * DL + N
            v.memset(Er[:, 0:e1], 0.0)
            v.memset(Er[:, e1 : e1 + 1], 1.0)
            v.memset(Er[:, e1 + 1 : KE * DL], 0.0)
            # scheduler fence: init memsets must not reorder past DP steps
            tc.no_sync_barrier()

            # ---- D window precompute: one sheared subtract + Square per
            # window of DW diagonals; yields dq[k*DL + pos] = D[d0+k][pos]
            # over each diagonal's extended range [lo-1, hi+1]. ----
            def emit_dwin(widx, tagp):
                d0, d1 = wins[widx]
                nd = d1 - d0 + 1
                dsw = pool.tile([SPC, DW * DL], F32, tag="dwin", bufs=2)
                if d1 <= N + 1:  # lower half: elo = 0, EL_d = d+1
                    count = d1 + 1
                    o_ap = _mk_ap(dsw, 0, [[DL, nd], [1, count]])
                    t_in = _mk_ap(text, 0, [[0, nd], [1, count]])
                    p_in = _mk_ap(prev, N - d0 + 1, [[-1, nd], [1, count]])
                else:  # upper half: elo_d = d-N-1, EL_d = 2N+3-d
                    count = 2 * N + 3 - d0
                    elo0 = d0 - N - 1
                    o_ap = _mk_ap(dsw, elo0, [[DL + 1, nd], [1, count]])
                    t_in = _mk_ap(text, elo0, [[1, nd], [1, count]])
                    p_in = _mk_ap(prev, 0, [[0, nd], [1, count]])
                v.tensor_tensor(o_ap, t_in, p_in, op=OP.subtract)
                s.activation(o_ap, o_ap, AF.Square)  # square in place
                return dsw

            # window index for a diagonal
            wof = {}
            for i, (d0, d1) in enumerate(wins):
                for d in range(d0, d1 + 1):
                    wof[d] = i

            # ---- forward wavefront ----
            fwin = {}  # widx -> (dqw tile, d0)
            fwin[0] = (emit_dwin(0, "f"), wins[0][0])
            for d in range(2, 2 * N + 1):
                wi = wof[d]
                if d == wins[wi][0] and wi + 1 < NW:
                    fwin[wi + 1] = (emit_dwin(wi + 1, "f"), wins[wi + 1][0])
                dqw, wd0 = fwin[wi]
                lo, hi = _rng(d)
                L = hi - lo + 1
                rb = d * DL
                ko = (d - wd0) * DL
                p2s = R[:, (d - 2) * DL + lo - 1 : (d - 2) * DL + lo - 1 + L]
                p1s = R[:, (d - 1) * DL + lo - 1 : (d - 1) * DL + lo - 1 + L]
                p1 = R[:, (d - 1) * DL + lo : (d - 1) * DL + lo + L]

                m1 = pool.tile([SPC, DL], F32, tag="f_m1", bufs=4)
                mm = pool.tile([SPC, DL], F32, tag="f_mm", bufs=4)
                stk = pool.tile([SPC, 3 * DL], F32, tag="f_stk", bufs=4)
                est = pool.tile([SPC, 3 * DL], BF16, tag="f_est", bufs=4)
                sm0 = pool.tile([SPC, DL], BF16, tag="f_sm0", bufs=4)
                ssm = pool.tile([SPC, DL], BF16, tag="f_ssm", bufs=4)
                lnb = pool.tile([SPC, DL], F32, tag="f_lnb", bufs=4)
                dm = pool.tile([SPC, DL], F32, tag="f_dm", bufs=4)

                v.tensor_tensor(m1[:, 0:L], p2s, p1s, op=OP.min)
                v.tensor_tensor(mm[:, 0:L], m1[:, 0:L], p1, op=OP.min)
                v.tensor_sub(stk[:, 0:L], p2s, mm[:, 0:L])
                v.tensor_sub(stk[:, DL : DL + L], p1s, mm[:, 0:L])
                v.tensor_sub(stk[:, 2 * DL : 2 * DL + L], p1, mm[:, 0:L])
                s.activation(
                    _mk_ap(est, 0, [[DL, 3], [1, L]]),
                    _mk_ap(stk, 0, [[DL, 3], [1, L]]),
                    AF.Exp,
                    scale=-INVG,
                )
                v.tensor_add(sm0[:, 0:L], est[:, 0:L], est[:, DL : DL + L])
                v.tensor_add(ssm[:, 0:L], sm0[:, 0:L], est[:, 2 * DL : 2 * DL + L])
                s.activation(lnb[:, 0:L], ssm[:, 0:L], AF.Ln)
                v.tensor_add(dm[:, 0:L], dqw[:, ko + lo : ko + lo + L], mm[:, 0:L])
                v.scalar_tensor_tensor(
                    R[:, rb + lo : rb + lo + L],
                    lnb[:, 0:L],
                    -GAMMA,
                    dm[:, 0:L],
                    op0=OP.mult,
                    op1=OP.add,
                )

            # ---- backward (E recursion + Omega accumulation) ----
            bwin = {}
            bwin[NW - 1] = (emit_dwin(NW - 1, "b"), wins[NW - 1][0])

            def s_prep(dd):
                """S[dd] = R[dd] - D[dd] over extended range [lo-1, hi+1]."""
                wi = wof[dd]
                if wi not in bwin:
                    bwin[wi] = (emit_dwin(wi, "b"), wins[wi][0])
                dqw, wd0 = bwin[wi]
                ko = (dd - wd0) * DL
                lo, hi = _rng(dd)
                elo = lo - 1
                EL = hi - lo + 3
                sb = (dd % KS) * DL
                v.tensor_sub(
                    Sr[:, sb + elo : sb + elo + EL],
                    R[:, dd * DL + elo : dd * DL + elo + EL],
                    dqw[:, ko + elo : ko + elo + EL],
                )

            s_prep(2 * N)

            step_i = 0
            for d in range(2 * N - 1, 1, -1):
                lo, hi = _rng(d)
                L = hi - lo + 1
                if d + 1 < 2 * N:
                    # prefetch the next window before it is first needed
                    wi = wof[d + 1]
                    if wi not in bwin and wi - 1 >= 0:
                        pass
                    s_prep(d + 1)
                    if d + 1 == wins[wof[d + 1]][0] and wof[d + 1] - 1 >= 0:
                        wj = wof[d + 1] - 1
                        if wj not in bwin:
                            bwin[wj] = (emit_dwin(wj, "b"), wins[wj][0])
                S1 = Sr[:, ((d + 1) % KS) * DL : ((d + 1) % KS) * DL + DL]
                S2 = Sr[:, ((d + 2) % KS) * DL : ((d + 2) % KS) * DL + DL]
                E1 = Er[:, ((d + 1) % KE) * DL : ((d + 1) % KE) * DL + DL]
                E2 = Er[:, ((d + 2) % KE) * DL : ((d + 2) % KE) * DL + DL]
                Ed = Er[:, (d % KE) * DL : (d % KE) * DL + DL]
                Rd = R[:, d * DL + lo : d * DL + lo + L]

                bst = pool.tile([SPC, 3 * DL], F32, tag="b_bst", bufs=4)
                bes = pool.tile([SPC, 3 * DL], BF16, tag="b_bes", bufs=4)
                pst = pool.tile([SPC, 3 * DL], BF16, tag="b_pst", bufs=4)
                pt0 = pool.tile([SPC, DL], BF16, tag="b_pt0", bufs=4)
                scr = pool.tile([SPC, DL], F32, tag="b_scr", bufs=4)

                v.tensor_sub(bst[:, 0:L], S1[:, lo + 1 : lo + 1 + L], Rd)
                v.tensor_sub(bst[:, DL : DL + L], S1[:, lo : lo + L], Rd)
                v.tensor_sub(bst[:, 2 * DL : 2 * DL + L], S2[:, lo + 1 : lo + 1 + L], Rd)
                s.activation(
                    _mk_ap(bes, 0, [[DL, 3], [1, L]]),
                    _mk_ap(bst, 0, [[DL, 3], [1, L]]),
                    AF.Exp,
                    scale=INVG,
                )
                v.tensor_mul(pst[:, 0:L], bes[:, 0:L], E1[:, lo + 1 : lo + 1 + L])
                v.tensor_mul(pst[:, DL : DL + L], bes[:, DL : DL + L], E1[:, lo : lo + L])
                v.tensor_mul(
                    pst[:, 2 * DL : 2 * DL + L],
                    bes[:, 2 * DL : 2 * DL + L],
                    E2[:, lo + 1 : lo + 1 + L],
                )
                v.tensor_add(pt0[:, 0:L], pst[:, 0:L], pst[:, DL : DL + L])
                v.tensor_add(Ed[:, lo : lo + L], pt0[:, 0:L], pst[:, 2 * DL : 2 * DL + L])
                # Omega: weight (2*idx - d)^2 = sqtab read at stride 2;
                # STT out = Ed * sqt, accum_out -> accv column for this step
                c0 = 2 * lo - d + SQOFF
                v.scalar_tensor_tensor(
                    scr[:, 0:L],
                    Ed[:, lo : lo + L],
                    1.0,
                    sqt[:, c0 : c0 + 2 * L : 2],
                    op0=OP.bypass,
                    op1=OP.mult,
                    accum_out=accv[:, step_i : step_i + 1],
                )
                step_i += 1

            v.tensor_reduce(
                accs[:, 0:1], accv[:, 0:step_i], axis=mybir.AxisListType.X, op=OP.add
            )
            v.tensor_copy(outp[:, 0:1], R[:, 2 * N * DL + N : 2 * N * DL + N + 1])
            v.tensor_copy(outp[:, 1:2], accs[:, 0:1])
            nc.sync.dma_start(out_d[:, :], outp[:, :])

    if legalize:
        _split_multi_waits(nc)
    return nc


def _shard_inputs(input, target):
    p = np.transpose(np.asarray(input, np.float32), (0, 2, 3, 4, 1)).reshape(-1, T)
    t = np.transpose(np.asarray(target, np.float32), (0, 2, 3, 4, 1)).reshape(-1, T)
    in_maps = []
    for k in range(NCORES):
        sl = slice(k * SPC, (k + 1) * SPC)
        t_ext = np.full((SPC, TP), SENT, np.float32)
        t_ext[:, 1 : T + 1] = t[sl]
        p_rev = np.full((SPC, TP), SENT, np.float32)
        p_rev[:, 1 : T + 1] = p[sl][:, ::-1]
        in_maps.append({"t_ext": t_ext, "p_rev_ext": p_rev})
    return in_maps


def _reduce_outputs(results):
    ls = np.concatenate([r["out"][:, 0] for r in results])
    tacc = np.concatenate([r["out"][:, 1] for r in results])
    loss_shape = ls.mean(dtype=np.float64)
    loss_temporal = (tacc / (T * T)).mean(dtype=np.float64)
    loss = ALPHA * loss_shape + (1.0 - ALPHA) * loss_temporal
    return np.array([loss, loss_shape, loss_temporal], np.float32)


def kernel(input, target, _cache={}):
    if "nc" not in _cache:
        _cache["nc"] = build_nc()
    res = bass_utils.run_bass_kernel_spmd(
        _cache["nc"], _shard_inputs(input, target), core_ids=list(range(NCORES))
    )
    return _reduce_outputs(res.results)


# revision 21
# speedup vs baseline: 1.6691x; 1.6691x over previous
"""Trainium2 Bass kernel for the soft-DTW shape+temporal loss.

Problem: input/target (4, 128, 16, 4, 4) = (B, T, C, H, W). Each of the
B*C*H*W = 1024 spatial cells is an independent univariate series of length
T = 128. Per series: squared-L2 cost matrix D, soft-DTW forward DP value
R[N,N] (loss_shape), soft alignment path = dR[N,N]/dD (via the standard
backward E-recursion), temporal loss = sum(path * Omega)/T^2 with
Omega[i,j] = (i-j)^2.

Sharding: 1024 series / 8 cores = 128 series per core, one series per SBUF
partition. The DP runs along anti-diagonals (wavefront); diagonal cells are
laid out along the free dimension, so every wavefront step is a handful of
full-width vector ops. R is stored diagonal-major (slot width DL per
diagonal) for the whole forward sweep; the backward pass re-reads it.

Cost-matrix diagonals are precomputed in windows of W=8 diagonals with one
sheared 2D-access-pattern subtract + one sheared Square activation (instead
of two per-step ops); post-exp quantities (softmin exp terms, path weights,
E) are bf16 so their sums run in the DVE's 2x packed mode.

Host side shards the inputs, runs the same program SPMD on 8 cores, and
reduces the per-series results to the 3 scalar losses.
"""

import sys

for _p in ("/opt/trn_rl_repo",):
    if _p not in sys.path:
        sys.path.insert(0, _p)

import numpy as np

import concourse.bass as bass
import concourse.mybir as mybir
from concourse import bass_utils
from concourse.tile import TileContext

# ---- problem constants (hardcoded per contract) ----
B, T, C, H, W_ = 4, 128, 16, 4, 4
N = T
NCORES = 8
SPC = (B * C * H * W_) // NCORES  # 128 series per core
ALPHA = 0.5
GAMMA = 0.01
INVG = 1.0 / GAMMA
BIG = 1e8
SENT = 1.0e6  # sentinel pad; (x - SENT)^2 ~ 1e12 >> BIG kills boundary weights

DL = 140  # per-diagonal slot width (>= N + DW + 4 so window tails stay in-slot)
ND = 2 * N + 1  # diagonals 0..2N
KS = 4  # S ring depth
KE = 4  # E ring depth
SQOFF = 256  # sqtab column offset: col = (2*idx - d) + SQOFF
DW = 8  # D-precompute window size (diagonals per window)
TP = 140  # padded width of t/p_rev host arrays (sentinel tail for windows)

F32 = mybir.dt.float32
BF16 = mybir.dt.bfloat16
I32 = mybir.dt.int32
AF = mybir.ActivationFunctionType
OP = mybir.AluOpType


def _rng(d):
    """Valid idx range [lo, hi] of diagonal d (cells (i=idx, j=d-idx))."""
    return max(1, d - N), min(N, d - 1)


def _win_list():
    """Windows of diagonals, each entirely in the lower (d<=N+1) or upper
    half so the sheared access patterns stay affine."""
    wins = []
    d = 2
    while d <= N + 1:
        wins.append((d, min(d + DW - 1, N + 1)))
        d += DW
    d = N + 2
    while d <= 2 * N:
        wins.append((d, min(d + DW - 1, 2 * N)))
        d += DW
    return wins


def _split_multi_waits(nc):
    """walrus here rejects >1 sync wait per TPB instruction.

    Pass 1 (ACT only): drop self-engine waits that are provably satisfied
    by program order — the ACT instruction struct cannot carry 2 waits and
    NoOp carriers are rejected by the ACT codegen path.
    Pass 2: hoist remaining extra waits onto same-engine NoOp carriers.
    """
    pre_of = {
        mybir.EngineType.DVE: "DVE",
        mybir.EngineType.Activation: "Activation",
        mybir.EngineType.Pool: "Pool",
        mybir.EngineType.SP: "SP",
        mybir.EngineType.PE: "PE",
    }
    nsplit = 0
    inc = {}  # (engine, sem id) -> inc count so far, in block order
    tainted = set()
    for f in nc.m.functions:
        for bb in f.blocks:
            insts = list(bb.instructions)
            new = []
            changed = False
            for ins in insts:
                si = ins.sync_info
                eng = ins.engine
                pre = pre_of.get(eng)
                waits = list(si.on_wait) if si is not None and si.on_wait else []
                if (
                    waits
                    and pre is not None
                    and len(waits) > 1
                    and eng == mybir.EngineType.Activation
                ):
                    keep = [
                        w
                        for w in waits
                        if not (
                            w.sync_type == "semaphore"
                            and w.wait_mode == "sem-ge-imm"
                            and w.ant_name
                            and w.ant_name.split("_")[0] == pre
                            and w.id not in tainted
                            and w.wait_value <= inc.get((eng, w.id), 0)
                        )
                    ]
                else:
                    keep = waits
                if len(keep) > 1:
                    for w in keep[:-1]:
                        nsplit += 1
                        new.append(
                            mybir.InstNoOp(
                                name=f"wsplit-{nsplit}",
                                engine=eng,
                                sync_info=mybir.SyncInfo(on_wait=[w], on_update=[]),
                            )
                        )
                    keep = [keep[-1]]
                    changed = True
                if si is not None and len(keep) != len(waits):
                    ins.sync_info = mybir.SyncInfo(
                        on_wait=keep, on_update=list(si.on_update or [])
                    )
                    changed = True
                if si is not None and si.on_update:
                    for u in si.on_update:
                        if u.update_mode == "sem-inc":
                            inc[(eng, u.id)] = inc.get((eng, u.id), 0) + (
                                u.update_value or 0
                            )
                        else:
                            tainted.add(u.id)
                new.append(ins)
            if changed:
                bb.instructions = new
    return nsplit


def _mk_ap(tile_ap, off, axes):
    """Raw AP over a tile: axes = [[stride, count], ...] after the partition
    axis (which is taken from the tile)."""
    base = tile_ap[:, 0:1]
    return bass.AP(
        tensor=base.tensor, offset=off, ap=[[base.ap[0][0], SPC]] + axes
    )


def build_nc(legalize=True):
    nc = bass.Bass("TRN2", debug=False, num_devices=NCORES)
    t_ext_d = nc.dram_tensor("t_ext", [SPC, TP], F32, kind="ExternalInput")
    p_rev_d = nc.dram_tensor("p_rev_ext", [SPC, TP], F32, kind="ExternalInput")
    out_d = nc.dram_tensor("out", [SPC, 2], F32, kind="ExternalOutput")

    wins = _win_list()
    NW = len(wins)
    NB = 2 * N - 1 - 1  # number of backward steps (d = 2N-1 .. 2)

    with TileContext(nc) as tc:
        with tc.tile_pool(name="main", bufs=1) as pool:
            v = nc.vector
            s = nc.scalar

            # ---- persistent state ----
            R = pool.tile([SPC, ND * DL], F32, tag="R")
            text = pool.tile([SPC, TP], F32, tag="text")
            prev = pool.tile([SPC, TP], F32, tag="prev")
            sqi = pool.tile([SPC, 512], I32, tag="sqi")
            sqt = pool.tile([SPC, 512], F32, tag="sqt")
            Sr = pool.tile([SPC, KS * DL], F32, tag="Sr")
            Er = pool.tile([SPC, KE * DL], BF16, tag="Er")
            accv = pool.tile([SPC, 256], F32, tag="accv")
            accs = pool.tile([SPC, 1], F32, tag="accs")
            outp = pool.tile([SPC, 2], F32, tag="outp")

            nc.sync.dma_start(text[:, :], t_ext_d[:, :])
            nc.sync.dma_start(prev[:, :], p_rev_d[:, :])

            # sq table: sqt[col] = (col - SQOFF)^2, same in every partition
            nc.gpsimd.iota(sqi[:, :], pattern=[[1, 512]], base=0, channel_multiplier=0)
            nbias = pool.tile([SPC, 1], F32, tag="nbias")
            nc.gpsimd.memset(nbias[:, :], float(-SQOFF))
            s.activation(sqt[:, :], sqi[:, :], AF.Square, bias=nbias[:, 0:1])

            # ---- R boundary init (only slots ever read as BIG) ----
            v.memset(R[:, 1 : N + 2], BIG)
            v.memset(R[:, DL : DL + N + 2], BIG)  # diag 1
            v.memset(R[:, 0:1], 0.0)
            # column 0 of diags 2..N+1 (lo-1 boundary, lower half)
            v.memset(R[:, 2 * DL : (N + 2) * DL : DL], BIG)
            # lo-1 boundary, upper half: diag d >= N+2 at position d-N-1
            v.memset(R[:, (N + 2) * DL + 1 : ND * DL : DL + 1], BIG)
            # hi+1 boundary, lower half: diag d in 1..N at position d
            v.memset(R[:, (DL + 1) : (N + 1) * (DL + 1) : DL + 1], BIG)
            # hi+1 boundary, upper half: diag d >= N+1 at position N+1
            v.memset(R[:, (N + 1) * DL + N + 1 : ND * DL : DL], BIG)

            v.memset(Sr[:, :], -BIG)
            # E ring: all zeros except E[2N][N] = 1 (disjoint writes)
            e1 = ((2 * N) % KE) * DL + N
            v.memset(Er[:, 0:e1], 0.0)
            v.memset(Er[:, e1 : e1 + 1], 1.0)
            v.memset(Er[:, e1 + 1 : KE * DL], 0.0)
            # forward (m~, s) rings: R[d] = m~[d] - g*ln(s[d]); slot d%3.
            # diag 0: m~[0][0]=0, else BIG; diag 1: BIG; s = 1 everywhere.
            Mr = pool.tile([SPC, 3 * DL], F32, tag="Mr")
            sr = pool.tile([SPC, 3 * DL], BF16, tag="sr")
            v.memset(Mr[:, 0:1], 0.0)
            v.memset(Mr[:, 1 : 3 * DL], BIG)
            v.memset(sr[:, :], 1.0)
            # scheduler fence: init memsets must not reorder past DP steps
            tc.no_sync_barrier()

            # ---- D window precompute: one sheared subtract + Square per
            # window of DW diagonals; yields dq[k*DL + pos] = D[d0+k][pos]
            # over each diagonal's extended range [lo-1, hi+1]. ----
            def emit_dwin(widx, tagp):
                d0, d1 = wins[widx]
                nd = d1 - d0 + 1
                dsw = pool.tile([SPC, DW * DL], F32, tag="dwin", bufs=2)
                if d1 <= N + 1:  # lower half: elo = 0, EL_d = d+1
                    count = d1 + 1
                    o_ap = _mk_ap(dsw, 0, [[DL, nd], [1, count]])
                    t_in = _mk_ap(text, 0, [[0, nd], [1, count]])
                    p_in = _mk_ap(prev, N - d0 + 1, [[-1, nd], [1, count]])
                else:  # upper half: elo_d = d-N-1, EL_d = 2N+3-d
                    count = 2 * N + 3 - d0
                    elo0 = d0 - N - 1
                    o_ap = _mk_ap(dsw, elo0, [[DL + 1, nd], [1, count]])
                    t_in = _mk_ap(text, elo0, [[1, nd], [1, count]])
                    p_in = _mk_ap(prev, 0, [[0, nd], [1, count]])
                v.tensor_tensor(o_ap, t_in, p_in, op=OP.subtract)
                s.activation(o_ap, o_ap, AF.Square)  # square in place
                return dsw

            # window index for a diagonal
            wof = {}
            for i, (d0, d1) in enumerate(wins):
                for d in range(d0, d1 + 1):
                    wof[d] = i

            # ---- forward wavefront ----
            fwin = {}  # widx -> (dqw tile, d0)
            fwin[0] = (emit_dwin(0, "f"), wins[0][0])
            for d in range(2, 2 * N + 1):
                wi = wof[d]
                if d == wins[wi][0] and wi + 1 < NW:
                    fwin[wi + 1] = (emit_dwin(wi + 1, "f"), wins[wi + 1][0])
                dqw, wd0 = fwin[wi]
                lo, hi = _rng(d)
                L = hi - lo + 1
                rb = d * DL
                ko = (d - wd0) * DL
                sa = ((d - 2) % 3) * DL  # ring slot of diag d-2
                sb = ((d - 1) % 3) * DL  # ring slot of diag d-1
                sc = (d % 3) * DL  # ring slot of diag d
                p2s = Mr[:, sa + lo - 1 : sa + lo - 1 + L]
                p1 = Mr[:, sb + lo : sb + lo + L]

                m1 = pool.tile([SPC, DL], F32, tag="f_m1", bufs=4)
                mm = pool.tile([SPC, DL], F32, tag="f_mm", bufs=4)
                stk = pool.tile([SPC, 3 * DL], F32, tag="f_stk", bufs=4)
                est = pool.tile([SPC, 3 * DL], BF16, tag="f_est", bufs=4)
                pstf = pool.tile([SPC, 3 * DL], BF16, tag="f_pst", bufs=4)
                sm0 = pool.tile([SPC, DL], BF16, tag="f_sm0", bufs=4)
                lnb = pool.tile([SPC, DL], F32, tag="f_lnb", bufs=4)

                v.tensor_tensor(
                    m1[:, 0:L], p2s, Mr[:, sb + lo - 1 : sb + lo - 1 + L], op=OP.min
                )
                v.tensor_tensor(mm[:, 0:L], m1[:, 0:L], p1, op=OP.min)
                # args: seg0 = p2s - M; segs 1,2 = (m~[d-1] at lo-1, lo) - M
                v.tensor_sub(stk[:, 0:L], p2s, mm[:, 0:L])
                v.tensor_tensor(
                    _mk_ap(stk, DL, [[DL, 2], [1, L]]),
                    bass.AP(
                        tensor=Mr[:, 0:1].tensor,
                        offset=sb + lo - 1,
                        ap=[[Mr[:, 0:1].ap[0][0], SPC], [1, 2], [1, L]],
                    ),
                    _mk_ap(mm, 0, [[0, 2], [1, L]]),
                    op=OP.subtract,
                )
                s.activation(
                    _mk_ap(est, 0, [[DL, 3], [1, L]]),
                    _mk_ap(stk, 0, [[DL, 3], [1, L]]),
                    AF.Exp,
                    scale=-INVG,
                )
                # terms: e_k * s_k  (bf16)
                v.tensor_mul(
                    pstf[:, 0:L], est[:, 0:L], sr[:, sa + lo - 1 : sa + lo - 1 + L]
                )
                v.tensor_tensor(
                    _mk_ap(pstf, DL, [[DL, 2], [1, L]]),
                    _mk_ap(est, DL, [[DL, 2], [1, L]]),
                    bass.AP(
                        tensor=sr[:, 0:1].tensor,
                        offset=sb + lo - 1,
                        ap=[[sr[:, 0:1].ap[0][0], SPC], [1, 2], [1, L]],
                    ),
                    op=OP.mult,
                )
                v.tensor_add(sm0[:, 0:L], pstf[:, 0:L], pstf[:, DL : DL + L])
                v.tensor_add(
                    sr[:, sc + lo : sc + lo + L],
                    sm0[:, 0:L],
                    pstf[:, 2 * DL : 2 * DL + L],
                )
                # m~[d] = D + M
                v.tensor_add(
                    Mr[:, sc + lo : sc + lo + L],
                    dqw[:, ko + lo : ko + lo + L],
                    mm[:, 0:L],
                )
                # exact R[d] = m~[d] - g*ln(s[d])  (off the min-chain)
                s.activation(lnb[:, 0:L], sr[:, sc + lo : sc + lo + L], AF.Ln)
                v.scalar_tensor_tensor(
                    R[:, rb + lo : rb + lo + L],
                    lnb[:, 0:L],
                    -GAMMA,
                    Mr[:, sc + lo : sc + lo + L],
                    op0=OP.mult,
                    op1=OP.add,
                )
                if d == 2:
                    # slot 0 is reused by diag 3+: restore the BIG boundary
                    # over the special m~[0][0] = 0 entry after its last read
                    v.tensor_scalar_mul(
                        Mr[:, 0:1], nc.const_aps.tensor(1.0, (SPC, 1), F32), BIG
                    )
                if d % 16 == 0:
                    # renormalise the ring pair to (R[d], 1) so s stays bounded
                    v.tensor_copy(Mr[:, sc + lo : sc + lo + L], R[:, rb + lo : rb + lo + L])
                    v.tensor_scalar_mul(
                        sr[:, sc + lo : sc + lo + L],
                        nc.const_aps.tensor(1.0, (SPC, L), BF16),
                        1.0,
                    )

            # ---- backward (E recursion + Omega accumulation) ----
            bwin = {}
            bwin[NW - 1] = (emit_dwin(NW - 1, "b"), wins[NW - 1][0])

            def s_prep(dd):
                """S[dd] = R[dd] - D[dd] over extended range [lo-1, hi+1]."""
                wi = wof[dd]
                if wi not in bwin:
                    bwin[wi] = (emit_dwin(wi, "b"), wins[wi][0])
                dqw, wd0 = bwin[wi]
                ko = (dd - wd0) * DL
                lo, hi = _rng(dd)
                elo = lo - 1
                EL = hi - lo + 3
                sb = (dd % KS) * DL
                v.tensor_sub(
                    Sr[:, sb + elo : sb + elo + EL],
                    R[:, dd * DL + elo : dd * DL + elo + EL],
                    dqw[:, ko + elo : ko + elo + EL],
                )

            s_prep(2 * N)

            step_i = 0
            for d in range(2 * N - 1, 1, -1):
                lo, hi = _rng(d)
                L = hi - lo + 1
                if d + 1 < 2 * N:
                    # prefetch the next window before it is first needed
                    wi = wof[d + 1]
                    if wi not in bwin and wi - 1 >= 0:
                        pass
                    s_prep(d + 1)
                    if d + 1 == wins[wof[d + 1]][0] and wof[d + 1] - 1 >= 0:
                        wj = wof[d + 1] - 1
                        if wj not in bwin:
                            bwin[wj] = (emit_dwin(wj, "b"), wins[wj][0])
                S1 = Sr[:, ((d + 1) % KS) * DL : ((d + 1) % KS) * DL + DL]
                S2 = Sr[:, ((d + 2) % KS) * DL : ((d + 2) % KS) * DL + DL]
                E1 = Er[:, ((d + 1) % KE) * DL : ((d + 1) % KE) * DL + DL]
                E2 = Er[:, ((d + 2) % KE) * DL : ((d + 2) % KE) * DL + DL]
                Ed = Er[:, (d % KE) * DL : (d % KE) * DL + DL]
                Rd = R[:, d * DL + lo : d * DL + lo + L]

                bst = pool.tile([SPC, 3 * DL], F32, tag="b_bst", bufs=4)
                bes = pool.tile([SPC, 3 * DL], BF16, tag="b_bes", bufs=4)
                pst = pool.tile([SPC, 3 * DL], BF16, tag="b_pst", bufs=4)
                pt0 = pool.tile([SPC, DL], BF16, tag="b_pt0", bufs=4)
                scr = pool.tile([SPC, DL], F32, tag="b_scr", bufs=4)

                # segs 0,1 read S1 at idx+1, idx -> paired 2D AP (stride -1)
                v.tensor_tensor(
                    _mk_ap(bst, 0, [[DL, 2], [1, L]]),
                    bass.AP(
                        tensor=S1.tensor,
                        offset=S1.offset + lo + 1,
                        ap=[[S1.ap[0][0], SPC], [-1, 2], [1, L]],
                    ),
                    _mk_ap(Rd, Rd.offset, [[0, 2], [1, L]]),
                    op=OP.subtract,
                )
                v.tensor_sub(bst[:, 2 * DL : 2 * DL + L], S2[:, lo + 1 : lo + 1 + L], Rd)
                s.activation(
                    _mk_ap(bes, 0, [[DL, 3], [1, L]]),
                    _mk_ap(bst, 0, [[DL, 3], [1, L]]),
                    AF.Exp,
                    scale=INVG,
                )
                v.tensor_tensor(
                    _mk_ap(pst, 0, [[DL, 2], [1, L]]),
                    _mk_ap(bes, 0, [[DL, 2], [1, L]]),
                    bass.AP(
                        tensor=E1.tensor,
                        offset=E1.offset + lo + 1,
                        ap=[[E1.ap[0][0], SPC], [-1, 2], [1, L]],
                    ),
                    op=OP.mult,
                )
                v.tensor_mul(
                    pst[:, 2 * DL : 2 * DL + L],
                    bes[:, 2 * DL : 2 * DL + L],
                    E2[:, lo + 1 : lo + 1 + L],
                )
                v.tensor_add(pt0[:, 0:L], pst[:, 0:L], pst[:, DL : DL + L])
                v.tensor_add(Ed[:, lo : lo + L], pt0[:, 0:L], pst[:, 2 * DL : 2 * DL + L])
                # Omega: weight (2*idx - d)^2 = sqtab read at stride 2;
                # STT out = Ed * sqt, accum_out -> accv column for this step
                c0 = 2 * lo - d + SQOFF
                v.scalar_tensor_tensor(
                    scr[:, 0:L],
                    Ed[:, lo : lo + L],
                    1.0,
                    sqt[:, c0 : c0 + 2 * L : 2],
                    op0=OP.bypass,
                    op1=OP.mult,
                    accum_out=accv[:, step_i : step_i + 1],
                )
                step_i += 1

            v.tensor_reduce(
                accs[:, 0:1], accv[:, 0:step_i], axis=mybir.AxisListType.X, op=OP.add
            )
            v.tensor_copy(outp[:, 0:1], R[:, 2 * N * DL + N : 2 * N * DL + N + 1])
            v.tensor_copy(outp[:, 1:2], accs[:, 0:1])
            nc.sync.dma_start(out_d[:, :], outp[:, :])

    if legalize:
        _split_multi_waits(nc)
    return nc


def _shard_inputs(input, target):
    p = np.transpose(np.asarray(input, np.float32), (0, 2, 3, 4, 1)).reshape(-1, T)
    t = np.transpose(np.asarray(target, np.float32), (0, 2, 3, 4, 1)).reshape(-1, T)
    in_maps = []
    for k in range(NCORES):
        sl = slice(k * SPC, (k + 1) * SPC)
        t_ext = np.full((SPC, TP), SENT, np.float32)
        t_ext[:, 1 : T + 1] = t[sl]
        p_rev = np.full((SPC, TP), SENT, np.float32)
        p_rev[:, 1 : T + 1] = p[sl][:, ::-1]
        in_maps.append({"t_ext": t_ext, "p_rev_ext": p_rev})
    return in_maps


def _reduce_outputs(results):
    ls = np.concatenate([r["out"][:, 0] for r in results])
    tacc = np.concatenate([r["out"][:, 1] for r in results])
    loss_shape = ls.mean(dtype=np.float64)
    loss_temporal = (tacc / (T * T)).mean(dtype=np.float64)
    loss = ALPHA * loss_shape + (1.0 - ALPHA) * loss_temporal
    return np.array([loss, loss_shape, loss_temporal], np.float32)


def kernel(input, target, _cache={}):
    if "nc" not in _cache:
        _cache["nc"] = build_nc()
    res = bass_utils.run_bass_kernel_spmd(
        _cache["nc"], _shard_inputs(input, target), core_ids=list(range(NCORES))
    )
    return _reduce_outputs(res.results)
